# revision 2
# baseline (speedup 1.0000x reference)
"""MemoryEfficientAttention on 8 TRN2 NeuronCores.

Full inputs in, full output out. Sharding: data-parallel over batch (2) x
tensor-parallel over heads (16 heads -> 4 heads/core). Each core computes
qkv projection for its heads, flash-style attention, and a partial output
projection over its 256 head-dims; the host sums the 4 partial projections
per batch and adds the bias.

All matmuls run as float32r (TF32-like, 1 cycle/row at N>=256 vs 4 for
fp32), measured ~2e-4 max rel err per matmul on HW.

Device layouts (T = transposed so the contraction dim is on partitions):
  xT  [1024, 2048]  x[b]^T                      (rhs of q/k, lhsT of v)
  wqT/wkT/wvT [1024, 256]  qkv_w slices^T       (lhsT of q/k, rhs of v)
  pwT [256, 1024]   proj_w column-slice^T       (rhs of proj)
  q^T/k^T computed as [d, n] (head-dim on partitions) so S^T = k^T-block
  matmuls need no transposes; V computed as [n, d]; PV matmul folds the
  softmax denominator via a ones column appended to V (Z lands on psum
  partition 64); normalization = reciprocal + gpsimd partition_broadcast +
  one DVE multiply, applied before the output projection.
"""

import numpy as np

B, N, C = 2, 2048, 1024
H, HD = 16, 64
NCORES = 8
TPG = 4              # tensor-parallel cores per batch
HPC = H // TPG       # 4 heads per core
D = HPC * HD         # 256 local head dims
KO = C // 128        # 8 contraction subtiles of the model dim
NB = N // 128        # 16 token blocks
MB = N // 128        # 16 key blocks
NT = 1024            # query-tile width in attention
NTC = N // NT
SCALE = HD ** -0.5

_state = {}


def _build_nc():
    import concourse.bass as bass
    import concourse.tile as tile
    import concourse.mybir as mybir
    from concourse import bacc

    f32 = mybir.dt.float32
    f32r = mybir.dt.float32r
    Exp = mybir.ActivationFunctionType.Exp
    mult = mybir.AluOpType.mult

    nc = bacc.Bacc("TRN2", target_bir_lowering=False, debug=False,
                   num_devices=NCORES)

    xT_d = nc.dram_tensor("xT", [C, N], f32r, kind="ExternalInput")
    wqT_d = nc.dram_tensor("wqT", [C, D], f32r, kind="ExternalInput")
    wkT_d = nc.dram_tensor("wkT", [C, D], f32r, kind="ExternalInput")
    wvT_d = nc.dram_tensor("wvT", [C, D], f32r, kind="ExternalInput")
    pwT_d = nc.dram_tensor("pwT", [D, C], f32r, kind="ExternalInput")
    ones_d = nc.dram_tensor("ones", [NB * HPC], f32r, kind="ExternalInput")
    y_d = nc.dram_tensor("y", [N, C], f32, kind="ExternalOutput")

    with tile.TileContext(nc) as tc:
        with (
            tc.tile_pool(name="big", bufs=1) as big,
            tc.tile_pool(name="work", bufs=2) as work,
            tc.tile_pool(name="outp", bufs=2) as outp,
            tc.tile_pool(name="ps_mm", bufs=2, space="PSUM") as ps_mm,
            tc.tile_pool(name="ps_s", bufs=2, space="PSUM") as ps_s,
            tc.tile_pool(name="ps_o", bufs=1, space="PSUM") as ps_o,
        ):
            xt = big.tile([128, KO, N], f32r, tag="xt")
            wq = big.tile([128, KO, D], f32r, tag="wq")
            wk = big.tile([128, KO, D], f32r, tag="wk")
            wv = big.tile([128, KO, D], f32r, tag="wv")
            pw = big.tile([128, D // 128, C], f32r, tag="pw")
            qt = [big.tile([128, N], f32r, tag=f"qt{t}", name=f"qt{t}")
                  for t in range(2)]
            kt = [big.tile([128, N], f32r, tag=f"kt{t}", name=f"kt{t}")
                  for t in range(2)]
            vt = big.tile([128, NB, HPC * (HD + 1)], f32r, tag="vt")
            ot = [big.tile([128, N], f32r, tag=f"ot{t}", name=f"ot{t}")
                  for t in range(2)]

            # ---- loads ----
            nc.sync.dma_start(
                wq[:], wqT_d.ap().rearrange("(ko p) d -> p ko d", p=128))
            nc.sync.dma_start(
                wk[:], wkT_d.ap().rearrange("(ko p) d -> p ko d", p=128))
            nc.sync.dma_start(
                wv[:], wvT_d.ap().rearrange("(ko p) d -> p ko d", p=128))
            nc.sync.dma_start(
                pw[:], pwT_d.ap().rearrange("(t p) e -> p t e", p=128))
            for ch in range(4):
                s = slice(ch * 512, (ch + 1) * 512)
                nc.sync.dma_start(
                    xt[:, :, s],
                    xT_d.ap()[:, s].rearrange("(ko p) n -> p ko n", p=128))
            # ones column of vt, broadcast from DRAM across partitions
            vt4 = vt[:].rearrange("p nb (h c) -> p nb h c", c=HD + 1)
            ones_src = bass.AP(
                tensor=ones_d.ap().tensor, offset=0,
                ap=[[0, 128], [HPC, NB], [1, HPC], [0, 1]])
            nc.sync.dma_start(vt4[:, :, :, HD:HD + 1], ones_src)

            # ---- qkv projection ----
            # q^T / k^T in [d, n] layout: lhsT = w slice, rhs = xT
            for w, dst in ((wq, qt), (wk, kt)):
                for t in range(2):
                    dsl = slice(t * 128, (t + 1) * 128)
                    for ch in range(4):
                        nsl = slice(ch * 512, (ch + 1) * 512)
                        pm = ps_mm.tile([128, 512], f32, tag="mm")
                        for ko in range(KO):
                            nc.tensor.matmul(
                                pm[:], w[:, ko, dsl], xt[:, ko, nsl],
                                start=(ko == 0), stop=(ko == KO - 1))
                        nc.vector.tensor_copy(dst[t][:, nsl], pm[:])
            # V in [n, d] layout: lhsT = xT block, rhs = wv
            for nb in range(NB):
                bsl = slice(nb * 128, (nb + 1) * 128)
                pm = ps_mm.tile([128, 512], f32, tag="mm")
                for ko in range(KO):
                    nc.tensor.matmul(
                        pm[:, :D], xt[:, ko, bsl], wv[:, ko, :],
                        start=(ko == 0), stop=(ko == KO - 1))
                nc.vector.tensor_copy(
                    vt4[:, nb, :, 0:HD],
                    pm[:, :D].rearrange("p (h c) -> p h c", c=HD))

            # ---- attention + projection ----
            for nt in range(NTC):
                qsl = slice(nt * NT, (nt + 1) * NT)
                for h in range(HPC):
                    t, hi = divmod(h, 2)
                    psl = slice(hi * 64, (hi + 1) * 64)
                    po = ps_o.tile([HD + 1, NT], f32, tag="po")
                    for mb in range(MB):
                        msl = slice(mb * 128, (mb + 1) * 128)
                        psb = ps_s.tile([128, NT], f32, tag="ps")
                        for sc in range(NT // 512):
                            ssl = slice(sc * 512, (sc + 1) * 512)
                            nc.tensor.matmul(
                                psb[:, ssl],
                                kt[t][psl, msl],
                                qt[t][psl, nt * NT + sc * 512:
                                      nt * NT + (sc + 1) * 512],
                                start=True, stop=True)
                        eb = work.tile([128, NT], f32r, tag="eb")
                        nc.scalar.activation(
                            out=eb[:], in_=psb[:], func=Exp, scale=SCALE)
                        for sc in range(NT // 512):
                            ssl = slice(sc * 512, (sc + 1) * 512)
                            nc.tensor.matmul(
                                po[:, ssl],
                                vt4[:, mb, h, :],
                                eb[:, ssl],
                                start=(mb == 0), stop=(mb == MB - 1))
                    # normalize: O^T[dh, n] * (1/Z[n])
                    rz = work.tile([1, NT], f32, tag="rz")
                    nc.vector.reciprocal(rz[:], po[HD:HD + 1, :])
                    rzb = work.tile([64, NT], f32, tag="rzb")
                    nc.gpsimd.partition_broadcast(rzb[:], rz[:])
                    if hi == 0:
                        nc.vector.tensor_tensor(
                            ot[t][0:64, qsl], po[0:HD, :], rzb[:], mult)
                    else:
                        otmp = work.tile([64, NT], f32r, tag="otmp")
                        nc.vector.tensor_tensor(
                            otmp[:], po[0:HD, :], rzb[:], mult)
                        nc.sync.dma_start(ot[t][64:128, qsl], otmp[:])

                # partial output projection for this query tile
                for nb in range(nt * NT // 128, (nt + 1) * NT // 128):
                    bsl = slice(nb * 128, (nb + 1) * 128)
                    for ech in range(2):
                        esl = slice(ech * 512, (ech + 1) * 512)
                        py = ps_mm.tile([128, 512], f32, tag="mm")
                        for t in range(2):
                            nc.tensor.matmul(
                                py[:], ot[t][:, bsl], pw[:, t, esl],
                                start=(t == 0), stop=(t == 1))
                        yb = outp.tile([128, 512], f32, tag="yb")
                        nc.vector.tensor_copy(yb[:], py[:])
                        nc.sync.dma_start(y_d.ap()[bsl, esl], yb[:])

    nc.compile()
    return nc


def _shard_inputs(x, qkv_w, proj_w):
    """Per-core input maps. Core c: batch c//4, heads 4*(c%4)..4*(c%4)+3."""
    ones = np.ones(NB * HPC, np.float32)
    in_maps = []
    for c in range(NCORES):
        b, g = divmod(c, TPG)
        dsl = slice(g * D, (g + 1) * D)
        in_maps.append({
            "xT": np.ascontiguousarray(x[b].T),
            "wqT": np.ascontiguousarray(qkv_w[dsl, :].T),
            "wkT": np.ascontiguousarray(qkv_w[C:2 * C][dsl, :].T),
            "wvT": np.ascontiguousarray(qkv_w[2 * C:][dsl, :].T),
            "pwT": np.ascontiguousarray(proj_w[:, dsl].T),
            "ones": ones,
        })
    return in_maps


def _get_runner():
    """Cached jitted 8-core SPMD runner (avoids re-jit per call)."""
    if "runner" in _state:
        return _state["runner"]
    import jax
    import concourse.mybir as mybir
    from concourse import bass2jax

    nc = _state.get("nc")
    if nc is None:
        nc = _state["nc"] = _build_nc()
    bass2jax.install_neuronx_cc_hook()

    partition_name = (nc.partition_id_tensor.name
                      if nc.partition_id_tensor else None)
    in_names, out_names, out_avals, zero_shapes = [], [], [], []
    for alloc in nc.m.functions[0].allocations:
        if not isinstance(alloc, mybir.MemoryLocationSet):
            continue
        name = alloc.memorylocations[0].name
        if alloc.kind == "ExternalInput":
            if name != partition_name:
                in_names.append(name)
        elif alloc.kind == "ExternalOutput":
            shape = tuple(alloc.tensor_shape)
            dtype = mybir.dt.np(alloc.dtype)
            out_names.append(name)
            out_avals.append(jax.core.ShapedArray(shape, dtype))
            zero_shapes.append((shape, dtype))
    n_params = len(in_names)
    all_in_names = list(in_names) + list(out_names)
    if partition_name is not None:
        all_in_names.append(partition_name)
    donate = tuple(range(n_params, n_params + len(out_names)))

    def _body(*args):
        operands = list(args)
        if partition_name is not None:
            operands.append(bass2jax.partition_id_tensor())
        outs = bass2jax._bass_exec_p.bind(
            *operands,
            out_avals=tuple(out_avals),
            in_names=tuple(all_in_names),
            out_names=tuple(out_names),
            lowering_input_output_aliases=(),
            sim_require_finite=True,
            sim_require_nnan=True,
            nc=nc,
        )
        return tuple(outs)

    devices = jax.devices()[:NCORES]
    mesh = bass2jax.Mesh(np.asarray(devices), ("core",))
    spec = (bass2jax.PartitionSpec("core"),)
    sharded = jax.jit(
        bass2jax.shard_map(
            _body, mesh=mesh,
            in_specs=spec * (n_params + len(out_names)),
            out_specs=spec * len(out_names),
            check_rep=False),
        donate_argnums=donate, keep_unused=True)

    def run(in_maps):
        concat_in = [
            np.concatenate([np.asarray(m[name]) for m in in_maps], axis=0)
            for name in in_names
        ]
        concat_zeros = [
            np.zeros((NCORES * s[0], *s[1:]), dt) for s, dt in zero_shapes
        ]
        out_arrs = sharded(*concat_in, *concat_zeros)
        return [
            {name: np.asarray(out_arrs[i]).reshape(NCORES, *out_avals[i].shape)[c]
             for i, name in enumerate(out_names)}
            for c in range(NCORES)
        ]

    _state["runner"] = run
    return run


def _combine(results, proj_b):
    """Sum the 4 tensor-parallel partial projections per batch, add bias."""
    out = np.empty((B, N, C), np.float32)
    for b in range(B):
        acc = results[b * TPG + 0]["y"].astype(np.float32).copy()
        for g in range(1, TPG):
            acc += results[b * TPG + g]["y"]
        out[b] = acc + proj_b[None, :]
    return out


def kernel(x, qkv_w, proj_w, proj_b):
    x = np.asarray(x, np.float32)
    qkv_w = np.asarray(qkv_w, np.float32)
    proj_w = np.asarray(proj_w, np.float32)
    proj_b = np.asarray(proj_b, np.float32)
    run = _get_runner()
    results = run(_shard_inputs(x, qkv_w, proj_w))
    return _combine(results, proj_b)


def run_traced(x, qkv_w, proj_w, proj_b, **trace_kwargs):
    """Profiling path for test.py: full run_bass_kernel_spmd with trace."""
    from concourse.bass_utils import run_bass_kernel_spmd
    nc = _state.get("nc")
    if nc is None:
        nc = _state["nc"] = _build_nc()
    in_maps = _shard_inputs(
        np.asarray(x, np.float32), np.asarray(qkv_w, np.float32),
        np.asarray(proj_w, np.float32))
    res = run_bass_kernel_spmd(
        nc, in_maps, core_ids=list(range(NCORES)), trace=True, **trace_kwargs)
    out = _combine(res.results, np.asarray(proj_b, np.float32))
    return out, res


# revision 6
# speedup vs baseline: 10155.9260x; 10155.9260x over previous
"""MemoryEfficientAttention on 8 TRN2 NeuronCores.

Full inputs in, full output out. Sharding: data-parallel over batch (2) x
tensor-parallel over heads (16 heads -> 4 heads/core). Each core computes
qkv projection for its heads, flash-style attention, and a partial output
projection over its 256 head-dims; the host sums the 4 partial projections
per batch and adds the bias.

All matmuls run as float32r (TF32-like, 1 cycle/row at N>=256 vs 4 for
fp32), measured ~2e-4 max rel err per matmul on HW.

Device layouts (T = transposed so the contraction dim is on partitions):
  xT  [1024, 2048]  x[b]^T                      (rhs of q/k, lhsT of v)
  wqT/wkT/wvT [1024, 256]  qkv_w slices^T       (lhsT of q/k, rhs of v)
  pwT [256, 1024]   proj_w column-slice^T       (rhs of proj)
  q^T/k^T computed as [d, n] (head-dim on partitions) so S^T = k^T-block
  matmuls need no transposes; V computed as [n, d]; PV matmul folds the
  softmax denominator via a ones column appended to V (Z lands on psum
  partition 64); normalization = reciprocal + gpsimd partition_broadcast +
  one DVE multiply, applied before the output projection.
"""

import numpy as np

B, N, C = 2, 2048, 1024
H, HD = 16, 64
NCORES = 8
TPG = 4              # tensor-parallel cores per batch
HPC = H // TPG       # 4 heads per core
D = HPC * HD         # 256 local head dims
KO = C // 128        # 8 contraction subtiles of the model dim
NB = N // 128        # 16 token blocks
MB = N // 128        # 16 key blocks
NT = 1024            # query-tile width in attention
NTC = N // NT
SCALE = HD ** -0.5

_state = {}


def _build_nc(reps=1):
    import concourse.bass as bass
    import concourse.tile as tile
    import concourse.mybir as mybir
    from concourse import bacc

    f32 = mybir.dt.float32
    f32r = mybir.dt.float32r
    Exp = mybir.ActivationFunctionType.Exp
    mult = mybir.AluOpType.mult

    nc = bacc.Bacc("TRN2", target_bir_lowering=False, debug=False,
                   num_devices=NCORES)

    xT_d = nc.dram_tensor("xT", [C, N], f32r, kind="ExternalInput")
    wqT_d = nc.dram_tensor("wqT", [C, D], f32r, kind="ExternalInput")
    wkT_d = nc.dram_tensor("wkT", [C, D], f32r, kind="ExternalInput")
    wvT_d = nc.dram_tensor("wvT", [C, D], f32r, kind="ExternalInput")
    pwT_d = nc.dram_tensor("pwT", [D, C], f32r, kind="ExternalInput")
    ones_d = nc.dram_tensor("ones", [NB * HPC], f32r, kind="ExternalInput")
    y_d = nc.dram_tensor("y", [N, C], f32, kind="ExternalOutput")

    with tile.TileContext(nc) as tc:
        with (
            tc.tile_pool(name="big", bufs=1) as big,
            tc.tile_pool(name="work", bufs=2) as work,
            tc.tile_pool(name="outp", bufs=2) as outp,
            tc.tile_pool(name="ps_mm", bufs=2, space="PSUM") as ps_mm,
            tc.tile_pool(name="ps_s", bufs=2, space="PSUM") as ps_s,
            tc.tile_pool(name="ps_o", bufs=1, space="PSUM") as ps_o,
        ):
            xt = big.tile([128, KO, N], f32r, tag="xt")
            wq = big.tile([128, KO, D], f32r, tag="wq")
            wk = big.tile([128, KO, D], f32r, tag="wk")
            wv = big.tile([128, KO, D], f32r, tag="wv")
            pw = big.tile([128, D // 128, C], f32r, tag="pw")
            qt = [big.tile([128, N], f32r, tag=f"qt{t}", name=f"qt{t}")
                  for t in range(2)]
            kt = [big.tile([128, N], f32r, tag=f"kt{t}", name=f"kt{t}")
                  for t in range(2)]
            vt = big.tile([128, NB, HPC * (HD + 1)], f32r, tag="vt")
            ot = [big.tile([128, N], f32r, tag=f"ot{t}", name=f"ot{t}")
                  for t in range(2)]
            vt4 = vt[:].rearrange("p nb (h c) -> p nb h c", c=HD + 1)

            def emit_body():
                # ---- loads ----
                nc.sync.dma_start(
                    wq[:], wqT_d.ap().rearrange("(ko p) d -> p ko d", p=128))
                nc.sync.dma_start(
                    wk[:], wkT_d.ap().rearrange("(ko p) d -> p ko d", p=128))
                nc.sync.dma_start(
                    wv[:], wvT_d.ap().rearrange("(ko p) d -> p ko d", p=128))
                nc.sync.dma_start(
                    pw[:], pwT_d.ap().rearrange("(t p) e -> p t e", p=128))
                for ch in range(4):
                    s = slice(ch * 512, (ch + 1) * 512)
                    nc.sync.dma_start(
                        xt[:, :, s],
                        xT_d.ap()[:, s].rearrange("(ko p) n -> p ko n", p=128))
                # ones column of vt, broadcast from DRAM across partitions
                ones_src = bass.AP(
                    tensor=ones_d.ap().tensor, offset=0,
                    ap=[[0, 128], [HPC, NB], [1, HPC], [0, 1]])
                nc.sync.dma_start(vt4[:, :, :, HD:HD + 1], ones_src)

                # ---- qkv projection ----
                # q^T / k^T in [d, n] layout: lhsT = w slice, rhs = xT
                for w, dst in ((wq, qt), (wk, kt)):
                    for t in range(2):
                        dsl = slice(t * 128, (t + 1) * 128)
                        for ch in range(4):
                            nsl = slice(ch * 512, (ch + 1) * 512)
                            pm = ps_mm.tile([128, 512], f32, tag="mm",
                                            name="pm")
                            for ko in range(KO):
                                nc.tensor.matmul(
                                    pm[:], w[:, ko, dsl], xt[:, ko, nsl],
                                    start=(ko == 0), stop=(ko == KO - 1))
                            nc.vector.tensor_copy(dst[t][:, nsl], pm[:])
                # V in [n, d] layout: lhsT = xT block, rhs = wv
                for nb in range(NB):
                    bsl = slice(nb * 128, (nb + 1) * 128)
                    pm = ps_mm.tile([128, 512], f32, tag="mm", name="pm")
                    for ko in range(KO):
                        nc.tensor.matmul(
                            pm[:, :D], xt[:, ko, bsl], wv[:, ko, :],
                            start=(ko == 0), stop=(ko == KO - 1))
                    nc.vector.tensor_copy(
                        vt4[:, nb, :, 0:HD],
                        pm[:, :D].rearrange("p (h c) -> p h c", c=HD))

                # ---- attention + projection ----
                for nt in range(NTC):
                    qsl = slice(nt * NT, (nt + 1) * NT)
                    for h in range(HPC):
                        t, hi = divmod(h, 2)
                        psl = slice(hi * 64, (hi + 1) * 64)
                        po = ps_o.tile([HD + 1, NT], f32, tag="po", name="po")
                        for mb in range(MB):
                            msl = slice(mb * 128, (mb + 1) * 128)
                            psb = ps_s.tile([128, NT], f32, tag="ps",
                                            name="psb")
                            for sc in range(NT // 512):
                                ssl = slice(sc * 512, (sc + 1) * 512)
                                nc.tensor.matmul(
                                    psb[:, ssl],
                                    kt[t][psl, msl],
                                    qt[t][psl, nt * NT + sc * 512:
                                          nt * NT + (sc + 1) * 512],
                                    start=True, stop=True)
                            eb = work.tile([128, NT], f32r, tag="eb",
                                           name="eb")
                            nc.scalar.activation(
                                out=eb[:], in_=psb[:], func=Exp, scale=SCALE)
                            for sc in range(NT // 512):
                                ssl = slice(sc * 512, (sc + 1) * 512)
                                nc.tensor.matmul(
                                    po[:, ssl],
                                    vt4[:, mb, h, :],
                                    eb[:, ssl],
                                    start=(mb == 0), stop=(mb == MB - 1))
                        # normalize: O^T[dh, n] * (1/Z[n])
                        rz = work.tile([1, NT], f32, tag="rz", name="rz")
                        nc.vector.reciprocal(rz[:], po[HD:HD + 1, :])
                        rzb = work.tile([64, NT], f32, tag="rzb", name="rzb")
                        nc.gpsimd.partition_broadcast(rzb[:], rz[:])
                        if hi == 0:
                            nc.vector.tensor_tensor(
                                ot[t][0:64, qsl], po[0:HD, :], rzb[:], mult)
                        else:
                            otmp = work.tile([64, NT], f32r, tag="otmp",
                                             name="otmp")
                            nc.vector.tensor_tensor(
                                otmp[:], po[0:HD, :], rzb[:], mult)
                            nc.sync.dma_start(ot[t][64:128, qsl], otmp[:])

                    # partial output projection for this query tile
                    for nb in range(nt * NT // 128, (nt + 1) * NT // 128):
                        bsl = slice(nb * 128, (nb + 1) * 128)
                        for ech in range(2):
                            esl = slice(ech * 512, (ech + 1) * 512)
                            py = ps_mm.tile([128, 512], f32, tag="mm",
                                            name="py")
                            for t in range(2):
                                nc.tensor.matmul(
                                    py[:], ot[t][:, bsl], pw[:, t, esl],
                                    start=(t == 0), stop=(t == 1))
                            yb = outp.tile([128, 512], f32, tag="yb",
                                           name="yb")
                            nc.vector.tensor_copy(yb[:], py[:])
                            nc.sync.dma_start(y_d.ap()[bsl, esl], yb[:])

            if reps == 1:
                emit_body()
            else:
                # device-side hardware loop: one dispatch, reps executions
                with tc.For_i(0, reps, 1):
                    emit_body()

    nc.compile()
    return nc


def _get_nc(reps=1):
    key = f"nc{reps}"
    if key not in _state:
        _state[key] = _build_nc(reps)
    return _state[key]


def _shard_inputs(x, qkv_w, proj_w):
    """Per-core input maps. Core c: batch c//4, heads 4*(c%4)..4*(c%4)+3."""
    ones = np.ones(NB * HPC, np.float32)
    in_maps = []
    for c in range(NCORES):
        b, g = divmod(c, TPG)
        dsl = slice(g * D, (g + 1) * D)
        in_maps.append({
            "xT": np.ascontiguousarray(x[b].T),
            "wqT": np.ascontiguousarray(qkv_w[dsl, :].T),
            "wkT": np.ascontiguousarray(qkv_w[C:2 * C][dsl, :].T),
            "wvT": np.ascontiguousarray(qkv_w[2 * C:][dsl, :].T),
            "pwT": np.ascontiguousarray(proj_w[:, dsl].T),
            "ones": ones,
        })
    return in_maps


def _make_runner(nc, donate=True):
    """Jitted 8-core SPMD runner for a built Bass module."""
    import jax
    import concourse.mybir as mybir
    from concourse import bass2jax

    bass2jax.install_neuronx_cc_hook()

    partition_name = (nc.partition_id_tensor.name
                      if nc.partition_id_tensor else None)
    in_names, out_names, out_avals, zero_shapes = [], [], [], []
    for alloc in nc.m.functions[0].allocations:
        if not isinstance(alloc, mybir.MemoryLocationSet):
            continue
        name = alloc.memorylocations[0].name
        if alloc.kind == "ExternalInput":
            if name != partition_name:
                in_names.append(name)
        elif alloc.kind == "ExternalOutput":
            shape = tuple(alloc.tensor_shape)
            dtype = mybir.dt.np(alloc.dtype)
            out_names.append(name)
            out_avals.append(jax.core.ShapedArray(shape, dtype))
            zero_shapes.append((shape, dtype))
    n_params = len(in_names)
    all_in_names = list(in_names) + list(out_names)
    if partition_name is not None:
        all_in_names.append(partition_name)
    donate_idx = tuple(range(n_params, n_params + len(out_names))) if donate \
        else ()

    def _body(*args):
        operands = list(args)
        if partition_name is not None:
            operands.append(bass2jax.partition_id_tensor())
        outs = bass2jax._bass_exec_p.bind(
            *operands,
            out_avals=tuple(out_avals),
            in_names=tuple(all_in_names),
            out_names=tuple(out_names),
            lowering_input_output_aliases=(),
            sim_require_finite=True,
            sim_require_nnan=True,
            nc=nc,
        )
        return tuple(outs)

    devices = jax.devices()[:NCORES]
    mesh = bass2jax.Mesh(np.asarray(devices), ("core",))
    spec = (bass2jax.PartitionSpec("core"),)
    sharded = jax.jit(
        bass2jax.shard_map(
            _body, mesh=mesh,
            in_specs=spec * (n_params + len(out_names)),
            out_specs=spec * len(out_names),
            check_rep=False),
        donate_argnums=donate_idx, keep_unused=True)

    meta = dict(in_names=in_names, out_names=out_names, out_avals=out_avals,
                zero_shapes=zero_shapes, mesh=mesh)
    return sharded, meta


def _get_runner():
    if "runner" in _state:
        return _state["runner"]
    nc = _get_nc(1)
    sharded, meta = _make_runner(nc, donate=True)

    def run(in_maps):
        concat_in = [
            np.concatenate([np.asarray(m[name]) for m in in_maps], axis=0)
            for name in meta["in_names"]
        ]
        concat_zeros = [
            np.zeros((NCORES * s[0], *s[1:]), dt)
            for s, dt in meta["zero_shapes"]
        ]
        out_arrs = sharded(*concat_in, *concat_zeros)
        out_avals = meta["out_avals"]
        return [
            {name: np.asarray(out_arrs[i]).reshape(
                NCORES, *out_avals[i].shape)[c]
             for i, name in enumerate(meta["out_names"])}
            for c in range(NCORES)
        ]

    _state["runner"] = run
    return run


def _combine(results, proj_b):
    """Sum the 4 tensor-parallel partial projections per batch, add bias."""
    out = np.empty((B, N, C), np.float32)
    for b in range(B):
        acc = results[b * TPG + 0]["y"].astype(np.float32).copy()
        for g in range(1, TPG):
            acc += results[b * TPG + g]["y"]
        out[b] = acc + proj_b[None, :]
    return out


def kernel(x, qkv_w, proj_w, proj_b):
    x = np.asarray(x, np.float32)
    qkv_w = np.asarray(qkv_w, np.float32)
    proj_w = np.asarray(proj_w, np.float32)
    proj_b = np.asarray(proj_b, np.float32)
    run = _get_runner()
    results = run(_shard_inputs(x, qkv_w, proj_w))
    return _combine(results, proj_b)


def make_timing_fn(reps, in_maps):
    """Device-resident, non-donating executor of the reps-times kernel.

    Returns fn() that launches one execution and blocks until done. Inputs
    (and dummy zero outputs) are placed on device once, so repeated calls
    measure dispatch + on-device execution only.
    """
    import jax
    from jax.sharding import NamedSharding
    from concourse import bass2jax

    nc = _get_nc(reps)
    sharded, meta = _make_runner(nc, donate=False)
    shd = NamedSharding(meta["mesh"], bass2jax.PartitionSpec("core"))
    dev_in = [
        jax.device_put(
            np.concatenate([np.asarray(m[name]) for m in in_maps], axis=0),
            shd)
        for name in meta["in_names"]
    ]
    dev_zero = [
        jax.device_put(np.zeros((NCORES * s[0], *s[1:]), dt), shd)
        for s, dt in meta["zero_shapes"]
    ]

    def fn():
        outs = sharded(*dev_in, *dev_zero)
        for o in outs:
            o.block_until_ready()
        return outs

    return fn


# revision 32
# speedup vs baseline: 15091.8444x; 1.4860x over previous
"""MemoryEfficientAttention on 8 TRN2 NeuronCores.

Full inputs in, full output out. Sharding: data-parallel over batch (2) x
tensor-parallel over heads (16 heads -> 4 heads/core). Each core computes
qkv projection for its heads, flash-style attention, and a partial output
projection over its 256 head-dims; the host sums the 4 partial projections
per batch and adds the bias.

All matmuls run as float32r (TF32-like, 1 cycle/row at N>=256 vs 4 for
fp32), measured ~2e-4 max rel err per matmul on HW.

Device layouts (T = transposed so the contraction dim is on partitions):
  xT  [1024, 2048]  x[b]^T                      (rhs of q/k, lhsT of v)
  wqT/wkT/wvT [1024, 256]  qkv_w slices^T       (lhsT of q/k, rhs of v)
  pwT [256, 1024]   proj_w column-slice^T       (rhs of proj)
  q^T/k^T computed as [d, n] (head-dim on partitions) so S^T = k^T-block
  matmuls need no transposes; V computed as [n, d]; PV matmul folds the
  softmax denominator via a ones column appended to V (Z lands on psum
  partition 64); normalization = reciprocal + gpsimd partition_broadcast +
  one DVE multiply, applied before the output projection.
"""

import numpy as np

B, N, C = 2, 2048, 1024
H, HD = 16, 64
NCORES = 8
TPG = 4              # tensor-parallel cores per batch
HPC = H // TPG       # 4 heads per core
D = HPC * HD         # 256 local head dims
KO = C // 128        # 8 contraction subtiles of the model dim
NB = N // 128        # 16 token blocks
MB = N // 128        # 16 key blocks
NT = 1024            # query-tile width in attention
NTC = N // NT
SCALE = HD ** -0.5

_state = {}


def _build_nc(reps=1, phase="full", dtype="f32r", opts=None):
    import concourse.bass as bass
    import concourse.tile as tile
    import concourse.mybir as mybir
    from concourse import bacc

    opts = {**dict(ps_s_bufs=2, ps_o_bufs=1, mm_bufs=2, eb_bufs=2,
                   outp_bufs=2, big_y=True, xt_rows=True, nt=1024,
                   early_free=True, v_first=True, pe_bcast=False),
            **(opts or {})}
    NT = opts["nt"]
    NTC = N // NT
    f32 = mybir.dt.float32
    f32r = mybir.dt.float32r
    mdt = f32r if dtype == "f32r" else mybir.dt.bfloat16
    Exp = mybir.ActivationFunctionType.Exp
    mult = mybir.AluOpType.mult

    nc = bacc.Bacc("TRN2", target_bir_lowering=False, debug=False,
                   num_devices=NCORES)

    xT_d = nc.dram_tensor("xT", [C, N], mdt, kind="ExternalInput")
    wqT_d = nc.dram_tensor("wqT", [C, D], mdt, kind="ExternalInput")
    wkT_d = nc.dram_tensor("wkT", [C, D], mdt, kind="ExternalInput")
    wvT_d = nc.dram_tensor("wvT", [C, D], mdt, kind="ExternalInput")
    pwT_d = nc.dram_tensor("pwT", [D, C], mdt, kind="ExternalInput")
    ones_d = nc.dram_tensor("ones", [NB * HPC], mdt, kind="ExternalInput")
    y_d = nc.dram_tensor("y", [N, C], f32, kind="ExternalOutput")

    with tile.TileContext(nc) as tc:
        with (
            tc.tile_pool(name="big", bufs=1) as big,
            tc.tile_pool(name="work", bufs=2) as work,
            tc.tile_pool(name="ebp", bufs=opts["eb_bufs"]) as ebp,
            tc.tile_pool(name="outp", bufs=opts["outp_bufs"]) as outp,
            tc.tile_pool(name="ps_mm", bufs=opts["mm_bufs"], space="PSUM") as ps_mm,
            tc.tile_pool(name="ps_s", bufs=opts["ps_s_bufs"], space="PSUM") as ps_s,
            tc.tile_pool(name="ps_o", bufs=opts["ps_o_bufs"], space="PSUM") as ps_o,
        ):
            xt = big.tile([128, KO, N], mdt, tag="xt")
            wq = big.tile([128, KO, D], mdt, tag="wq")
            wk = big.tile([128, KO, D], mdt, tag="wk")
            wv = big.tile([128, KO, D], mdt, tag="wv")
            pw = big.tile([128, D // 128, C], mdt, tag="pw")
            qt = [big.tile([128, N], mdt, tag=f"qt{t}", name=f"qt{t}")
                  for t in range(2)]
            kt = [big.tile([128, N], mdt, tag=f"kt{t}", name=f"kt{t}")
                  for t in range(2)]
            vt = big.tile([128, NB, HPC * (HD + 1)], mdt, tag="vt")
            ot = [big.tile([128, N], mdt, tag=f"ot{t}", name=f"ot{t}")
                  for t in range(2)]
            vt4 = vt[:].rearrange("p nb (h c) -> p nb h c", c=HD + 1)
            ebc = (big.tile([128, NT], mdt, tag="ebc", name="ebc")
                   if phase == "attn_noexp" else None)

            def emit_body():
                # ---- loads ----
                nc.sync.dma_start(
                    wq[:], wqT_d.ap().rearrange("(ko p) d -> p ko d", p=128))
                nc.sync.dma_start(
                    wk[:], wkT_d.ap().rearrange("(ko p) d -> p ko d", p=128))
                nc.sync.dma_start(
                    wv[:], wvT_d.ap().rearrange("(ko p) d -> p ko d", p=128))
                nc.sync.dma_start(
                    pw[:], pwT_d.ap().rearrange("(t p) e -> p t e", p=128))
                if opts["xt_rows"]:
                    for ko in range(KO):
                        nc.sync.dma_start(
                            xt[:, ko, :],
                            xT_d.ap()[ko * 128:(ko + 1) * 128, :])
                else:
                    for ch in range(4):
                        s = slice(ch * 512, (ch + 1) * 512)
                        nc.sync.dma_start(
                            xt[:, :, s],
                            xT_d.ap()[:, s].rearrange("(ko p) n -> p ko n", p=128))
                # ones column of vt: memset f32 staging + DVE cast-copy
                ones_sb = work.tile([128, NB * HPC], f32, tag="ones_sb",
                                    name="ones_sb", bufs=1)
                nc.vector.memset(ones_sb[:], 1.0)
                nc.vector.tensor_copy(
                    vt4[:, :, :, HD:HD + 1],
                    ones_sb[:].rearrange("p (nb h) -> p nb h", nb=NB
                                         ).unsqueeze(-1))

                # ---- qkv projection ----
                def emit_v():
                    # V in [n, d] layout: lhsT = xT block, rhs = wv
                    for nb in range(NB):
                        bsl = slice(nb * 128, (nb + 1) * 128)
                        pm = ps_mm.tile([128, 512], f32, tag="mm", name="pm")
                        for ko in range(KO):
                            nc.tensor.matmul(
                                pm[:, :D], xt[:, ko, bsl], wv[:, ko, :],
                                start=(ko == 0), stop=(ko == KO - 1))
                        nc.vector.tensor_copy(
                            vt4[:, nb, :, 0:HD],
                            pm[:, :D].rearrange("p (h c) -> p h c", c=HD))

                def emit_qk(order):
                    # q^T / k^T in [d, n] layout: lhsT = w slice, rhs = xT
                    for w, dst, t in order:
                        dsl = slice(t * 128, (t + 1) * 128)
                        for ch in range(4):
                            nsl = slice(ch * 512, (ch + 1) * 512)
                            pm = ps_mm.tile([128, 512], f32, tag="mm",
                                            name="pm")
                            for ko in range(KO):
                                nc.tensor.matmul(
                                    pm[:], w[:, ko, dsl], xt[:, ko, nsl],
                                    start=(ko == 0), stop=(ko == KO - 1))
                            nc.vector.tensor_copy(dst[t][:, nsl], pm[:])

                if opts["v_first"]:
                    emit_v()
                    emit_qk([(wq, qt, 0), (wk, kt, 0),
                             (wq, qt, 1), (wk, kt, 1)])
                else:
                    emit_qk([(wq, qt, 0), (wq, qt, 1),
                             (wk, kt, 0), (wk, kt, 1)])
                    emit_v()

                if phase == "qkv":
                    # dump q/k/v so nothing is dead-code-eliminated
                    yf = y_d.ap().rearrange("n c -> (n c)")
                    ofs = 0
                    for tl in (qt[0], qt[1], kt[0], kt[1]):
                        sz = 128 * N
                        nc.sync.dma_start(
                            yf[ofs:ofs + sz].rearrange("(p f) -> p f", p=128),
                            tl[:].bitcast(f32))
                        ofs += sz
                    sz = 128 * NB * HPC * (HD + 1)
                    nc.sync.dma_start(
                        yf[ofs:ofs + sz].rearrange("(p f) -> p f", p=128),
                        vt[:].bitcast(f32))
                    return

                # ---- attention + projection ----
                if phase == "attn_noexp":
                    nc.scalar.activation(
                        out=ebc[:], in_=qt[0][:, 0:NT], func=Exp, scale=SCALE)
                for nt in range(NTC):
                    qsl = slice(nt * NT, (nt + 1) * NT)
                    for h in range(HPC):
                        t, hi = divmod(h, 2)
                        psl = slice(hi * 64, (hi + 1) * 64)
                        po = ps_o.tile([HD + 1, NT], f32, tag="po", name="po")
                        for mb in range(MB):
                            msl = slice(mb * 128, (mb + 1) * 128)
                            psb = ps_s.tile([128, NT], f32, tag="ps",
                                            name="psb")
                            for sc in range(NT // 512):
                                ssl = slice(sc * 512, (sc + 1) * 512)
                                nc.tensor.matmul(
                                    psb[:, ssl],
                                    kt[t][psl, msl],
                                    qt[t][psl, nt * NT + sc * 512:
                                          nt * NT + (sc + 1) * 512],
                                    start=True, stop=True)
                            if phase == "attn_noexp":
                                eb = ebc
                            else:
                                eb = ebp.tile([128, NT], mdt, tag="eb",
                                              name="eb")
                                nc.scalar.activation(
                                    out=eb[:], in_=psb[:], func=Exp,
                                    scale=SCALE)
                            for sc in range(NT // 512):
                                ssl = slice(sc * 512, (sc + 1) * 512)
                                nc.tensor.matmul(
                                    po[:, ssl],
                                    vt4[:, mb, h, :],
                                    eb[:, ssl],
                                    start=(mb == 0), stop=(mb == MB - 1))
                        # normalize: O^T[dh, n] * (1/Z[n])
                        rz = work.tile([1, NT], f32, tag="rz", name="rz")
                        if opts["early_free"]:
                            # one copy frees po for the next group; the
                            # 3-hop normalize chain then runs off-path
                            poc = work.tile([HD + 1, NT], f32, tag="poc",
                                            name="poc", bufs=1)
                            nc.vector.tensor_copy(poc[:], po[:])
                            src_o = poc[0:HD, :]
                            nc.vector.reciprocal(rz[:], poc[HD:HD + 1, :])
                        else:
                            src_o = po[0:HD, :]
                            nc.vector.reciprocal(rz[:], po[HD:HD + 1, :])
                        if opts["pe_bcast"]:
                            # broadcast 1/Z across partitions via K=1 matmul
                            rzp = ps_mm.tile([128, 512], f32, tag="mm",
                                             name="rzp")
                            for sc in range(NT // 512):
                                nc.tensor.matmul(
                                    rzp[0:64, 0:512],
                                    ones_sb[0:1, 0:64],
                                    rz[:, sc * 512:(sc + 1) * 512],
                                    start=True, stop=True)
                                # copy to sbuf half (DVE)
                                if sc == 0:
                                    rzb = work.tile([64, NT], f32, tag="rzb",
                                                    name="rzb")
                                nc.vector.tensor_copy(
                                    rzb[:, sc * 512:(sc + 1) * 512],
                                    rzp[0:64, 0:512])
                        else:
                            rzb = work.tile([64, NT], f32, tag="rzb",
                                            name="rzb")
                            nc.gpsimd.partition_broadcast(rzb[:], rz[:])
                        nc.vector.tensor_tensor(
                            ot[t][psl, qsl], src_o, rzb[:], mult)

                    if phase in ("attn", "attn_noexp"):
                        continue
                    # partial output projection for this query tile
                    for nb in range(nt * NT // 128, (nt + 1) * NT // 128):
                        bsl = slice(nb * 128, (nb + 1) * 128)
                        ybig = (outp.tile([128, C], f32, tag="ybig",
                                          name="ybig")
                                if opts["big_y"] else None)
                        for ech in range(2):
                            esl = slice(ech * 512, (ech + 1) * 512)
                            py = ps_mm.tile([128, 512], f32, tag="mm",
                                            name="py")
                            for t in range(2):
                                nc.tensor.matmul(
                                    py[:], ot[t][:, bsl], pw[:, t, esl],
                                    start=(t == 0), stop=(t == 1))
                            if opts["big_y"]:
                                nc.vector.tensor_copy(ybig[:, esl], py[:])
                            else:
                                yb = outp.tile([128, 512], f32, tag="yb",
                                               name="yb")
                                nc.vector.tensor_copy(yb[:], py[:])
                                nc.sync.dma_start(y_d.ap()[bsl, esl], yb[:])
                        if opts["big_y"]:
                            nc.sync.dma_start(y_d.ap()[bsl, :], ybig[:])

                if phase in ("attn", "attn_noexp"):
                    yf = y_d.ap().rearrange("n c -> (n c)")
                    for i, tl in enumerate(ot):
                        sz = 128 * N
                        nc.sync.dma_start(
                            yf[i * sz:(i + 1) * sz].rearrange(
                                "(p f) -> p f", p=128),
                            tl[:].bitcast(f32))

            if reps == 1:
                emit_body()
            else:
                # device-side hardware loop: one dispatch, reps executions
                with tc.For_i(0, reps, 1):
                    emit_body()

    nc.compile()
    return nc


def _get_nc(reps=1, phase="full", dtype="f32r", opts=None):
    key = f"nc{reps}-{phase}-{dtype}-{sorted((opts or {}).items())}"
    if key not in _state:
        _state[key] = _build_nc(reps, phase, dtype, opts)
    return _state[key]


def _shard_inputs(x, qkv_w, proj_w, dtype="f32r"):
    """Per-core input maps. Core c: batch c//4, heads 4*(c%4)..4*(c%4)+3."""
    if dtype == "f32r":
        cast = lambda a: np.ascontiguousarray(a, np.float32)
    else:
        import ml_dtypes
        cast = lambda a: np.ascontiguousarray(a).astype(ml_dtypes.bfloat16)
    ones = cast(np.ones(NB * HPC, np.float32))
    in_maps = []
    for c in range(NCORES):
        b, g = divmod(c, TPG)
        dsl = slice(g * D, (g + 1) * D)
        in_maps.append({
            "xT": cast(x[b].T),
            "wqT": cast(qkv_w[dsl, :].T),
            "wkT": cast(qkv_w[C:2 * C][dsl, :].T),
            "wvT": cast(qkv_w[2 * C:][dsl, :].T),
            "pwT": cast(proj_w[:, dsl].T),
            "ones": ones,
        })
    return in_maps


def _make_runner(nc, donate=True):
    """Jitted 8-core SPMD runner for a built Bass module."""
    import jax
    import concourse.mybir as mybir
    from concourse import bass2jax

    bass2jax.install_neuronx_cc_hook()

    partition_name = (nc.partition_id_tensor.name
                      if nc.partition_id_tensor else None)
    in_names, out_names, out_avals, zero_shapes = [], [], [], []
    for alloc in nc.m.functions[0].allocations:
        if not isinstance(alloc, mybir.MemoryLocationSet):
            continue
        name = alloc.memorylocations[0].name
        if alloc.kind == "ExternalInput":
            if name != partition_name:
                in_names.append(name)
        elif alloc.kind == "ExternalOutput":
            shape = tuple(alloc.tensor_shape)
            dtype = mybir.dt.np(alloc.dtype)
            out_names.append(name)
            out_avals.append(jax.core.ShapedArray(shape, dtype))
            zero_shapes.append((shape, dtype))
    n_params = len(in_names)
    all_in_names = list(in_names) + list(out_names)
    if partition_name is not None:
        all_in_names.append(partition_name)
    donate_idx = tuple(range(n_params, n_params + len(out_names))) if donate \
        else ()

    def _body(*args):
        operands = list(args)
        if partition_name is not None:
            operands.append(bass2jax.partition_id_tensor())
        outs = bass2jax._bass_exec_p.bind(
            *operands,
            out_avals=tuple(out_avals),
            in_names=tuple(all_in_names),
            out_names=tuple(out_names),
            lowering_input_output_aliases=(),
            sim_require_finite=True,
            sim_require_nnan=True,
            nc=nc,
        )
        return tuple(outs)

    devices = jax.devices()[:NCORES]
    mesh = bass2jax.Mesh(np.asarray(devices), ("core",))
    spec = (bass2jax.PartitionSpec("core"),)
    sharded = jax.jit(
        bass2jax.shard_map(
            _body, mesh=mesh,
            in_specs=spec * (n_params + len(out_names)),
            out_specs=spec * len(out_names),
            check_rep=False),
        donate_argnums=donate_idx, keep_unused=True)

    meta = dict(in_names=in_names, out_names=out_names, out_avals=out_avals,
                zero_shapes=zero_shapes, mesh=mesh)
    return sharded, meta


def _get_runner():
    if "runner" in _state:
        return _state["runner"]
    nc = _get_nc(1)
    sharded, meta = _make_runner(nc, donate=True)

    def run(in_maps):
        concat_in = [
            np.concatenate([np.asarray(m[name]) for m in in_maps], axis=0)
            for name in meta["in_names"]
        ]
        concat_zeros = [
            np.zeros((NCORES * s[0], *s[1:]), dt)
            for s, dt in meta["zero_shapes"]
        ]
        out_arrs = sharded(*concat_in, *concat_zeros)
        out_avals = meta["out_avals"]
        return [
            {name: np.asarray(out_arrs[i]).reshape(
                NCORES, *out_avals[i].shape)[c]
             for i, name in enumerate(meta["out_names"])}
            for c in range(NCORES)
        ]

    _state["runner"] = run
    return run


def _combine(results, proj_b):
    """Sum the 4 tensor-parallel partial projections per batch, add bias."""
    out = np.empty((B, N, C), np.float32)
    for b in range(B):
        acc = results[b * TPG + 0]["y"].astype(np.float32).copy()
        for g in range(1, TPG):
            acc += results[b * TPG + g]["y"]
        out[b] = acc + proj_b[None, :]
    return out


def kernel(x, qkv_w, proj_w, proj_b):
    x = np.asarray(x, np.float32)
    qkv_w = np.asarray(qkv_w, np.float32)
    proj_w = np.asarray(proj_w, np.float32)
    proj_b = np.asarray(proj_b, np.float32)
    run = _get_runner()
    results = run(_shard_inputs(x, qkv_w, proj_w))
    return _combine(results, proj_b)


def make_timing_fn(reps, in_maps, phase="full", dtype="f32r", opts=None):
    """Device-resident, non-donating executor of the reps-times kernel.

    Returns fn() that launches one execution and blocks until done. Inputs
    (and dummy zero outputs) are placed on device once, so repeated calls
    measure dispatch + on-device execution only.
    """
    import jax
    from jax.sharding import NamedSharding
    from concourse import bass2jax

    nc = _get_nc(reps, phase, dtype, opts)
    sharded, meta = _make_runner(nc, donate=False)
    shd = NamedSharding(meta["mesh"], bass2jax.PartitionSpec("core"))
    dev_in = [
        jax.device_put(
            np.concatenate([np.asarray(m[name]) for m in in_maps], axis=0),
            shd)
        for name in meta["in_names"]
    ]
    dev_zero = [
        jax.device_put(np.zeros((NCORES * s[0], *s[1:]), dt), shd)
        for s, dt in meta["zero_shapes"]
    ]

    def fn():
        outs = sharded(*dev_in, *dev_zero)
        for o in outs:
            o.block_until_ready()
        return outs

    return fn
